# revision 51
# baseline (speedup 1.0000x reference)
"""Trainium2 Bass kernel for nn_AI4CFD (implicit advection solve, Richardson iteration).

Math: RE=0 in the reference, so L(u) = -(conv(u,w2)+b2) - (conv(u,w3)+b3) and the
scan body reduces to the direct f32 iteration
    residual_k = (u_k + DT*(conv(u_k, wc) + bc)) - u_old          wc = w2+w3, bc = b2+b3
    u_{k+1}    = u_k - residual_k  ==  (u_old - DT*bc) - DT*conv(u_k, wc)
In f32 the residual norm floors at rounding noise (~2e-5) far above TOL=1e-6, so the
convergence branch never fires: all 100 iterations run and final_res = |residual_99|.

Sharding: 1024 rows split 128/core across 8 cores (rows on SBUF partitions, cols on
the free dim).  conv = 3 banded-matmul accumulations on the PE array (one per column
offset) + single-row boundary fix-ups.  Ghost depth 2: one AllGather of 4 edge rows
per core every TWO iterations; the odd iteration's halo rows (u_{k+1}[-1], [128])
are computed locally in packed [128,8] form from the gathered depth-2 halo.  One
tiny AllReduce at the end for the residual norm.
"""

import os
import sys

import numpy as np

if "/opt/trn_rl_repo" not in sys.path:
    sys.path.insert(0, "/opt/trn_rl_repo")

N_CORES = 8
H = 1024
W = 1024
ROWS = H // N_CORES  # 128
DT = np.float32(0.05)
MAX_ITER = 100

_GRAPH_CACHE = {}


def _build_graph(n_iters):
    from concourse import bacc, bass, mybir, tile

    fp32 = mybir.dt.float32
    Alu = mybir.AluOpType

    nc = bacc.Bacc("TRN2", num_devices=N_CORES, debug=False,
                   target_bir_lowering=False)

    # ---- parameters (per-core shards; weight-derived tensors prepared on host) ----
    u_init = nc.declare_dram_parameter("u_init", [ROWS, W], fp32, isOutput=False)
    u0eff = nc.declare_dram_parameter("u0eff", [ROWS, W], fp32, isOutput=False)
    fix0 = nc.declare_dram_parameter("fix0", [ROWS, W], fp32, isOutput=False)
    est0 = nc.declare_dram_parameter("est0", [2, 1026], fp32, isOutput=False)
    zz = nc.declare_dram_parameter("zz", [2, 1026], fp32, isOutput=False)
    onesp = nc.declare_dram_parameter("onesp", [128, 1], fp32, isOutput=False)
    tmat = nc.declare_dram_parameter("tmat", [128, 3 * 128], fp32, isOutput=False)
    taps = nc.declare_dram_parameter("taps", [128, 6], fp32, isOutput=False)
    taps9 = nc.declare_dram_parameter("taps9", [128, 9], fp32, isOutput=False)
    u0ext = nc.declare_dram_parameter("u0ext", [128, 16], fp32, isOutput=False)
    # per-core one-hot row masks over the 32-row AllGather output, for the six
    # roles {u[-2], u[-1], u[0], u[127], u[128], u[129]}; zeros at the domain
    # boundary so the edge cores see zero halos automatically
    msel = nc.declare_dram_parameter("msel", [128, 6 * 10 * 32], fp32,
                                     isOutput=False)
    u_out = nc.declare_dram_parameter("u_out", [ROWS, W], fp32, isOutput=True)
    res_out = nc.declare_dram_parameter("res_out", [1, 1], fp32, isOutput=True)

    n_blocks = (n_iters + 1) // 2

    with tile.TileContext(nc) as tc:
        with (
            tc.tile_pool(name="const", bufs=1) as cpool,
            tc.tile_pool(name="u", bufs=2) as upool,
            tc.tile_pool(name="halo", bufs=2) as halopool,
            tc.tile_pool(name="fsm", bufs=2) as fpool,
            tc.tile_pool(name="psum", bufs=2, space="PSUM") as pspool,
            tc.tile_pool(name="fin", bufs=1) as finpool,
            tc.tile_pool(name="dram1", bufs=1, space="DRAM") as dpool1,
        ):
            # ---- constants into SBUF ----
            t_sb = cpool.tile([128, 3, 128], fp32, tag="tmat")
            nc.gpsimd.dma_start(t_sb[:], tmat[:].rearrange("k (t m) -> k t m", t=3))
            taps_sb = cpool.tile([128, 6], fp32, tag="taps")
            nc.gpsimd.dma_start(taps_sb[:], taps[:])
            taps9_sb = cpool.tile([128, 9], fp32, tag="taps9")
            nc.gpsimd.dma_start(taps9_sb[:], taps9[:])
            u0x_sb = cpool.tile([128, 2, 8], fp32, tag="u0ext")
            nc.gpsimd.dma_start(u0x_sb[:], u0ext[:].rearrange("p (s x) -> p s x", s=2))
            u0e_sb = cpool.tile([ROWS, W], fp32, tag="u0eff")
            nc.gpsimd.dma_start(u0e_sb[:], u0eff[:])
            msel_sb = cpool.tile([128, 6, 10, 32], fp32, tag="msel")
            nc.gpsimd.dma_start(msel_sb[:],
                                msel[:].rearrange("p (r x c) -> p r x c", r=6, x=10))
            ones = cpool.tile([128, 1], fp32, tag="ones")
            nc.gpsimd.dma_start(ones[:], onesp[:])

            # ---- DRAM bounce buffers (guard cols stay zero after init) ----
            arin_d = dpool1.tile([1, 512], fp32, tag="arin")
            arout_d = dpool1.tile([1, 512], fp32, tag="arout")
            nc.gpsimd.dma_start(arin_d[:], zz[0:1, 0:512])
            agin0 = dpool1.tile([4, 1026], fp32, tag="agin0")
            agin1 = dpool1.tile([4, 1026], fp32, tag="agin1")
            agout0 = dpool1.tile([32, 1026], fp32, tag="agout0")
            agout1 = dpool1.tile([32, 1026], fp32, tag="agout1")
            estg0 = dpool1.tile([2, 1026], fp32, tag="estg0")
            estg1 = dpool1.tile([2, 1026], fp32, tag="estg1")
            agins = [agin0, agin1]
            agouts = [agout0, agout1]
            estgs = [estg0, estg1]
            for ag in agins:
                nc.gpsimd.dma_start(ag[0:4:3, 0:1], zz[0:2, 0:1])
                nc.gpsimd.dma_start(ag[1:3, 0:1], zz[0:2, 0:1])
                nc.gpsimd.dma_start(ag[0:4:3, 1025:1026], zz[0:2, 1025:1026])
                nc.gpsimd.dma_start(ag[1:3, 1025:1026], zz[0:2, 1025:1026])
            for eg in estgs:
                nc.gpsimd.dma_start(eg[0:2, 0:1], zz[0:2, 0:1])
                nc.gpsimd.dma_start(eg[0:2, 1025:1026], zz[0:2, 1025:1026])

            # ---- initial state ----
            u_cur = upool.tile([ROWS, W], fp32, tag="u")
            nc.gpsimd.dma_start(u_cur[:], u_init[:])
            # boundary fix rows in a full-height tile (rows 1..126 stay zero)
            fixfull = cpool.tile([ROWS, W], fp32, tag="fixfull")
            nc.gpsimd.dma_start(fixfull[:], fix0[:])

            # touch t_sb on the PE first (single-wait discipline for LW)
            ps_warm = pspool.tile([1, 1], fp32, tag="warm")
            nc.tensor.matmul(ps_warm[:], t_sb[0:1, 0, 0:1], t_sb[0:1, 0, 0:1],
                             start=True, stop=True)

            def conv_iter(u_in):
                """One Richardson step: returns (psum, u_next)."""
                ps = pspool.tile([ROWS, W], fp32, tag="ps")
                for b0 in (0, 512):
                    b1 = b0 + 512
                    nc.tensor.matmul(ps[:, b0:b1], t_sb[:, 1], u_in[:, b0:b1],
                                     start=True, stop=False)
                    if b0 == 0:
                        nc.tensor.matmul(ps[:, 1:512], t_sb[:, 0], u_in[:, 0:511],
                                         start=False, stop=False)
                        nc.tensor.matmul(ps[:, 0:512], t_sb[:, 2], u_in[:, 1:513],
                                         start=False, stop=True)
                    else:
                        nc.tensor.matmul(ps[:, 512:1024], t_sb[:, 0],
                                         u_in[:, 511:1023], start=False, stop=False)
                        nc.tensor.matmul(ps[:, 512:1023], t_sb[:, 2],
                                         u_in[:, 513:1024], start=False, stop=True)
                u_nx = upool.tile([ROWS, W], fp32, tag="u")
                nc.vector.scalar_tensor_tensor(
                    u_nx[:], ps[:], -float(DT), u0e_sb[:], Alu.mult, Alu.add)
                nc.vector.tensor_tensor(u_nx[:], u_nx[:], fixfull[:], Alu.add)
                return ps, u_nx

            def fix_from(h_top, h_bot):
                """3-tap FIR of packed halo rows -> fixfull rows 0/127.
                h_top/h_bot are [128, 10] APs; taps are already * -DT."""
                fpk = fpool.tile([128, 2, 8], fp32, tag="fpk")
                for side, h in ((0, h_top), (1, h_bot)):
                    tb = 3 * side
                    nc.vector.tensor_scalar_mul(
                        fpk[:, side], h[:, 0:8], taps_sb[:, tb:tb + 1])
                    nc.vector.scalar_tensor_tensor(
                        fpk[:, side], h[:, 1:9], taps_sb[:, tb + 1:tb + 2],
                        fpk[:, side], Alu.mult, Alu.add)
                    nc.vector.scalar_tensor_tensor(
                        fpk[:, side], h[:, 2:10], taps_sb[:, tb + 2:tb + 3],
                        fpk[:, side], Alu.mult, Alu.add)
                nc.sync.dma_start(fixfull[0:1, :], fpk[:, 0])
                nc.scalar.dma_start(fixfull[ROWS - 1:ROWS, :], fpk[:, 1])

            psum_last = None
            u_prev = None
            done = False

            for b in range(n_blocks):
                k = 2 * b
                if b > 0:
                    # gathered depth-2 halo of u_k: select the six roles
                    agout_d = agouts[(b - 1) % 2]
                    halo = halopool.tile([128, 32, 10], fp32, tag="halo")
                    ag_t = agout_d[:].tensor
                    ag_off = agout_d[:].offset
                    nc.sync.dma_start(
                        halo[:, 0:16],
                        bass.AP(ag_t, ag_off, [(8, 128), (1026, 16), (1, 10)]))
                    nc.scalar.dma_start(
                        halo[:, 16:32],
                        bass.AP(ag_t, ag_off + 16 * 1026,
                                [(8, 128), (1026, 16), (1, 10)]))
                    hv = halo[:].transpose([0, 2, 1])  # [128, 10, 32] view
                    hsel = fpool.tile([128, 6, 10], fp32, tag="hsel")
                    for role in range(6):
                        htmp = halopool.tile([128, 10, 32], fp32, tag="htmp")
                        nc.vector.tensor_tensor(htmp[:], hv, msel_sb[:, role],
                                                Alu.mult)
                        nc.vector.tensor_reduce(hsel[:, role], htmp[:],
                                                mybir.AxisListType.X, Alu.add)
                    # fix rows for iteration k  (u_k[-1] -> top, u_k[128] -> bot)
                    fix_from(hsel[:, 1], hsel[:, 4])

                ps, u_next = conv_iter(u_cur)
                if k == n_iters - 1:
                    psum_last, u_prev, u_cur = ps, u_cur, u_next
                    done = True
                    break

                # ---- halo rows of u_{k+1} computed locally (packed form) ----
                if b == 0:
                    er = halopool.tile([128, 2, 10], fp32, tag="er")
                    nc.scalar.dma_start(
                        er[:], bass.AP(est0[:].tensor, 0,
                                       [(8, 128), (1026, 2), (1, 10)]))
                else:
                    acc = fpool.tile([128, 2, 8], fp32, tag="acc")
                    for side, roles in ((0, (0, 1, 2)), (1, (3, 4, 5))):
                        first = True
                        for w_row, role in zip((0, 1, 2), roles):
                            for bc_ in range(3):
                                scal = taps9_sb[:, 3 * w_row + bc_:
                                                3 * w_row + bc_ + 1]
                                src = hsel[:, role, bc_:bc_ + 8]
                                if first:
                                    nc.vector.tensor_scalar_mul(
                                        acc[:, side], src, scal)
                                    first = False
                                else:
                                    nc.vector.scalar_tensor_tensor(
                                        acc[:, side], src, scal, acc[:, side],
                                        Alu.mult, Alu.add)
                    epk = fpool.tile([128, 2, 8], fp32, tag="epk")
                    nc.vector.scalar_tensor_tensor(
                        epk[:, 0], acc[:, 0], -float(DT), u0x_sb[:, 0],
                        Alu.mult, Alu.add)
                    nc.vector.scalar_tensor_tensor(
                        epk[:, 1], acc[:, 1], -float(DT), u0x_sb[:, 1],
                        Alu.mult, Alu.add)
                    estg = estgs[b % 2]
                    nc.sync.dma_start(estg[0:1, 1:1025], epk[:, 0])
                    nc.scalar.dma_start(estg[1:2, 1:1025], epk[:, 1])
                    er = halopool.tile([128, 2, 10], fp32, tag="er")
                    nc.scalar.dma_start(
                        er[:], bass.AP(estg[:].tensor, estg[:].offset,
                                       [(8, 128), (1026, 2), (1, 10)]))
                # fix rows for iteration k+1
                fix_from(er[:, 0], er[:, 1])

                ps2b, u_next2 = conv_iter(u_next)
                if k + 1 == n_iters - 1:
                    psum_last, u_prev, u_cur = ps2b, u_next, u_next2
                    done = True
                    break

                # ---- pack + AllGather of u_{k+2}'s four edge rows ----
                agin_d = agins[b % 2]
                nc.sync.dma_start(agin_d[0:2, 1:1025], u_next2[0:2, :])
                nc.scalar.dma_start(agin_d[2:4, 1:1025],
                                    u_next2[ROWS - 2:ROWS, :])
                nc.gpsimd.collective_compute(
                    "AllGather",
                    Alu.bypass,
                    replica_groups=[list(range(N_CORES))],
                    ins=[agin_d[:].opt()],
                    outs=[agouts[b % 2][:].opt()],
                )
                u_cur = u_next2

            assert done

            # ---- final residual norm: res = (u_prev + DT*psum) - u0eff ----
            tt = finpool.tile([ROWS, W], fp32, tag="tt")
            nc.vector.scalar_tensor_tensor(
                tt[:], psum_last[:], float(DT), u_prev[:], Alu.mult, Alu.add)
            nc.vector.scalar_tensor_tensor(
                tt[:], fixfull[:], -1.0, tt[:], Alu.mult, Alu.add)
            res = finpool.tile([ROWS, W], fp32, tag="res")
            nc.vector.tensor_tensor(res[:], tt[:], u0e_sb[:], Alu.subtract)
            sq = finpool.tile([ROWS, W], fp32, tag="sq")
            part = finpool.tile([128, 1], fp32, tag="part")
            nc.vector.tensor_tensor(sq[:], res[:], res[:], Alu.mult)
            nc.vector.tensor_reduce(part[:], sq[:], mybir.AxisListType.X, Alu.add)
            ps2 = pspool.tile([1, 1], fp32, tag="ps2")
            nc.tensor.matmul(ps2[:], ones[:], part[:], start=True, stop=True)
            nsq = finpool.tile([1, 1], fp32, tag="nsq")
            nc.vector.tensor_copy(nsq[:], ps2[:])
            nc.sync.dma_start(arin_d[0:1, 0:1], nsq[:])
            nc.gpsimd.collective_compute(
                "AllReduce",
                Alu.add,
                replica_groups=[list(range(N_CORES))],
                ins=[arin_d[:].opt()],
                outs=[arout_d[:].opt()],
            )
            gn = finpool.tile([1, 1], fp32, tag="gn")
            nc.sync.dma_start(gn[:], arout_d[0:1, 0:1])
            fres = finpool.tile([1, 1], fp32, tag="fres")
            nc.scalar.sqrt(fres[:], gn[:])

            # ---- outputs ----
            nc.sync.dma_start(u_out[:], u_cur[:])
            nc.sync.dma_start(res_out[:], fres[:])

    nc.finalize()
    return nc


def _host_prep(u_old, w1, w2, w3, b1, b2, b3):
    """Prepare per-core input dicts (all host-side numpy, f32)."""
    u0 = np.asarray(u_old, dtype=np.float32).reshape(H, W)
    w2 = np.asarray(w2, dtype=np.float32).reshape(3, 3)
    w3 = np.asarray(w3, dtype=np.float32).reshape(3, 3)
    bc = np.float32(np.asarray(b2, dtype=np.float32).reshape(()) +
                    np.asarray(b3, dtype=np.float32).reshape(()))
    wc = (w2 + w3).astype(np.float32)

    u0eff = (u0 - DT * bc).astype(np.float32)

    # banded row-mixing matrices, one per column offset b:
    # T_b[i_in, i_out] = wc[i_in - i_out + 1, b]
    tmats = []
    for b in range(3):
        T = (np.diag(np.full(127, wc[0, b]), 1) +
             np.diag(np.full(128, wc[1, b]), 0) +
             np.diag(np.full(127, wc[2, b]), -1)).astype(np.float32)
        tmats.append(T)
    tmat = np.stack(tmats, axis=1).reshape(128, 3 * 128)  # [i_in, (b, i_out)]

    taps = np.zeros((128, 6), dtype=np.float32)
    taps[:, 0:3] = (-DT) * wc[0, :]
    taps[:, 3:6] = (-DT) * wc[2, :]
    taps9 = np.broadcast_to(wc.reshape(1, 9), (128, 9)).astype(np.float32).copy()

    def fir(h, r):  # 3-tap FIR of a row with wc[r, :] (f32, zero-padded)
        hp = np.zeros(W + 2, dtype=np.float32)
        hp[1:-1] = h
        return (wc[r, 0] * hp[0:W] + wc[r, 1] * hp[1:W + 1] +
                wc[r, 2] * hp[2:W + 2]).astype(np.float32)

    def row(i):  # u0 row with zero padding outside the domain
        if 0 <= i < H:
            return u0[i]
        return np.zeros(W, dtype=np.float32)

    def u1_row(i):  # u_1[i] = u0eff[i] - DT * conv(u0)[i]  (f32 host math)
        conv = fir(row(i - 1), 0) + fir(row(i), 1) + fir(row(i + 1), 2)
        base = u0eff[i] if 0 <= i < H else np.zeros(W, dtype=np.float32)
        return (base - DT * conv).astype(np.float32)

    def reshape_pk(rowvec):  # [1024] -> packed [128, 8]
        return rowvec.reshape(128, 8)

    in_maps = []
    for c in range(N_CORES):
        r0, r1 = c * ROWS, (c + 1) * ROWS
        htop = row(r0 - 1)
        hbot = row(r1)
        fix0 = np.zeros((ROWS, W), dtype=np.float32)
        fix0[0] = (-DT) * fir(htop, 0)
        fix0[ROWS - 1] = (-DT) * fir(hbot, 2)

        est0 = np.zeros((2, 1026), dtype=np.float32)
        if c > 0:
            est0[0, 1:1025] = u1_row(r0 - 1)
        if c < N_CORES - 1:
            est0[1, 1:1025] = u1_row(r1)

        u0x = np.zeros((128, 16), dtype=np.float32)
        if c > 0:
            u0x[:, 0:8] = reshape_pk(u0eff[r0 - 1])
        if c < N_CORES - 1:
            u0x[:, 8:16] = reshape_pk(u0eff[r1])

        # role masks over the 32 AG rows (AG row 4i+s = core i's
        # {r0, r1, r126, r127}[s])
        msel_m = np.zeros((6, 10, 32), dtype=np.float32)
        if c > 0:
            msel_m[0, :, 4 * (c - 1) + 2] = 1.0   # u_k[-2]
            msel_m[1, :, 4 * (c - 1) + 3] = 1.0   # u_k[-1]
            msel_m[2, :, 4 * c + 0] = 1.0         # u_k[0] (ext-row input only;
        if c < N_CORES - 1:                       # zero on the edge cores so
            msel_m[3, :, 4 * c + 3] = 1.0         # the ghost rows stay 0)
            msel_m[4, :, 4 * (c + 1) + 0] = 1.0   # u_k[128]
            msel_m[5, :, 4 * (c + 1) + 1] = 1.0   # u_k[129]

        in_maps.append({
            "u_init": np.ascontiguousarray(u0[r0:r1]),
            "u0eff": np.ascontiguousarray(u0eff[r0:r1]),
            "fix0": fix0,
            "est0": est0,
            "zz": np.zeros((2, 1026), dtype=np.float32),
            "onesp": np.ones((128, 1), dtype=np.float32),
            "tmat": tmat,
            "taps": taps,
            "taps9": taps9,
            "u0ext": u0x,
            "msel": np.broadcast_to(
                msel_m.reshape(1, 6 * 10 * 32), (128, 6 * 10 * 32)).copy(),
        })
    return in_maps


def kernel(u_old, w1, w2, w3, b1, b2, b3):
    from concourse.bass_utils import run_bass_kernel_spmd

    n_iters = MAX_ITER
    if n_iters not in _GRAPH_CACHE:
        _GRAPH_CACHE[n_iters] = _build_graph(n_iters)
    nc = _GRAPH_CACHE[n_iters]

    in_maps = _host_prep(u_old, w1, w2, w3, b1, b2, b3)
    out = run_bass_kernel_spmd(nc, in_maps, core_ids=list(range(N_CORES)))
    results = out.results

    u_full = np.concatenate([results[c]["u_out"] for c in range(N_CORES)], axis=0)
    u_full = u_full.reshape(1, 1, H, W).astype(np.float32)
    final_res = np.float32(results[0]["res_out"].reshape(())[()])
    return u_full, np.asarray(final_res, dtype=np.float32).reshape(())


# revision 55
# speedup vs baseline: 1.0094x; 1.0094x over previous
"""Trainium2 Bass kernel for nn_AI4CFD (implicit advection solve, Richardson iteration).

Math: RE=0 in the reference, so L(u) = -(conv(u,w2)+b2) - (conv(u,w3)+b3) and the
scan body reduces to the direct f32 iteration
    residual_k = (u_k + DT*(conv(u_k, wc) + bc)) - u_old          wc = w2+w3, bc = b2+b3
    u_{k+1}    = u_k - residual_k  ==  (u_old - DT*bc) - DT*conv(u_k, wc)
In f32 the residual norm floors at rounding noise (~2e-5) far above TOL=1e-6, so the
convergence branch never fires: all 100 iterations run and final_res = |residual_99|.

Sharding: 1024 rows split 128/core across 8 cores (rows on SBUF partitions, cols on
the free dim).  conv = 3 banded-matmul accumulations on the PE array (one per column
offset) + single-row boundary fix-ups.  Ghost depth 2: one AllGather of 4 edge rows
per core every TWO iterations; the odd iteration's halo rows (u_{k+1}[-1], [128])
are computed locally in packed [128,8] form from the gathered depth-2 halo.  One
tiny AllReduce at the end for the residual norm.
"""

import os
import sys

import numpy as np

if "/opt/trn_rl_repo" not in sys.path:
    sys.path.insert(0, "/opt/trn_rl_repo")

N_CORES = 8
H = 1024
W = 1024
ROWS = H // N_CORES  # 128
DT = np.float32(0.05)
MAX_ITER = 100

_GRAPH_CACHE = {}


def _build_graph(n_iters):
    from concourse import bacc, bass, mybir, tile

    fp32 = mybir.dt.float32
    Alu = mybir.AluOpType

    nc = bacc.Bacc("TRN2", num_devices=N_CORES, debug=False,
                   target_bir_lowering=False)

    # ---- parameters (per-core shards; weight-derived tensors prepared on host) ----
    u_init = nc.declare_dram_parameter("u_init", [ROWS, W], fp32, isOutput=False)
    u0eff = nc.declare_dram_parameter("u0eff", [ROWS, W], fp32, isOutput=False)
    fix0 = nc.declare_dram_parameter("fix0", [ROWS, W], fp32, isOutput=False)
    est0 = nc.declare_dram_parameter("est0", [2, 1026], fp32, isOutput=False)
    zz = nc.declare_dram_parameter("zz", [2, 1026], fp32, isOutput=False)
    onesp = nc.declare_dram_parameter("onesp", [128, 1], fp32, isOutput=False)
    tmat = nc.declare_dram_parameter("tmat", [128, 3 * 128], fp32, isOutput=False)
    taps = nc.declare_dram_parameter("taps", [128, 6], fp32, isOutput=False)
    taps9 = nc.declare_dram_parameter("taps9", [128, 9], fp32, isOutput=False)
    u0ext = nc.declare_dram_parameter("u0ext", [128, 16], fp32, isOutput=False)
    # per-core one-hot row masks over the 32-row AllGather output, for the six
    # roles {u[-2], u[-1], u[0], u[127], u[128], u[129]}; zeros at the domain
    # boundary so the edge cores see zero halos automatically
    msel = nc.declare_dram_parameter("msel", [128, 6 * 10 * 32], fp32,
                                     isOutput=False)
    u_out = nc.declare_dram_parameter("u_out", [ROWS, W], fp32, isOutput=True)
    res_out = nc.declare_dram_parameter("res_out", [1, 1], fp32, isOutput=True)

    n_blocks = (n_iters + 1) // 2

    with tile.TileContext(nc) as tc:
        with (
            tc.tile_pool(name="const", bufs=1) as cpool,
            tc.tile_pool(name="u", bufs=2) as upool,
            tc.tile_pool(name="halo", bufs=2) as halopool,
            tc.tile_pool(name="fsm", bufs=2) as fpool,
            tc.tile_pool(name="psum", bufs=2, space="PSUM") as pspool,
            tc.tile_pool(name="fin", bufs=1) as finpool,
            tc.tile_pool(name="dram1", bufs=1, space="DRAM") as dpool1,
        ):
            # ---- constants into SBUF ----
            t_sb = cpool.tile([128, 3, 128], fp32, tag="tmat")
            nc.gpsimd.dma_start(t_sb[:], tmat[:].rearrange("k (t m) -> k t m", t=3))
            taps_sb = cpool.tile([128, 6], fp32, tag="taps")
            nc.gpsimd.dma_start(taps_sb[:], taps[:])
            taps9_sb = cpool.tile([128, 9], fp32, tag="taps9")
            nc.gpsimd.dma_start(taps9_sb[:], taps9[:])
            u0x_sb = cpool.tile([128, 2, 8], fp32, tag="u0ext")
            nc.gpsimd.dma_start(u0x_sb[:], u0ext[:].rearrange("p (s x) -> p s x", s=2))
            u0e_sb = cpool.tile([ROWS, W], fp32, tag="u0eff")
            nc.gpsimd.dma_start(u0e_sb[:], u0eff[:])
            msel_sb = cpool.tile([128, 6, 10, 32], fp32, tag="msel")
            nc.gpsimd.dma_start(msel_sb[:],
                                msel[:].rearrange("p (r x c) -> p r x c", r=6, x=10))
            ones = cpool.tile([128, 1], fp32, tag="ones")
            nc.gpsimd.dma_start(ones[:], onesp[:])

            # ---- DRAM bounce buffers (guard cols stay zero after init) ----
            arin_d = dpool1.tile([1, 512], fp32, tag="arin")
            arout_d = dpool1.tile([1, 512], fp32, tag="arout")
            nc.gpsimd.dma_start(arin_d[:], zz[0:1, 0:512])
            agin0 = dpool1.tile([4, 1026], fp32, tag="agin0")
            agin1 = dpool1.tile([4, 1026], fp32, tag="agin1")
            agout0 = dpool1.tile([32, 1026], fp32, tag="agout0")
            agout1 = dpool1.tile([32, 1026], fp32, tag="agout1")
            estg0 = dpool1.tile([2, 1026], fp32, tag="estg0")
            estg1 = dpool1.tile([2, 1026], fp32, tag="estg1")
            agins = [agin0, agin1]
            agouts = [agout0, agout1]
            estgs = [estg0, estg1]
            for ag in agins:
                nc.gpsimd.dma_start(ag[0:4:3, 0:1], zz[0:2, 0:1])
                nc.gpsimd.dma_start(ag[1:3, 0:1], zz[0:2, 0:1])
                nc.gpsimd.dma_start(ag[0:4:3, 1025:1026], zz[0:2, 1025:1026])
                nc.gpsimd.dma_start(ag[1:3, 1025:1026], zz[0:2, 1025:1026])
            for eg in estgs:
                nc.gpsimd.dma_start(eg[0:2, 0:1], zz[0:2, 0:1])
                nc.gpsimd.dma_start(eg[0:2, 1025:1026], zz[0:2, 1025:1026])

            # ---- initial state ----
            u_cur = upool.tile([ROWS, W], fp32, tag="u")
            nc.gpsimd.dma_start(u_cur[:], u_init[:])
            # boundary fix rows in a full-height tile (rows 1..126 stay zero)
            fixfull = cpool.tile([ROWS, W], fp32, tag="fixfull")
            nc.gpsimd.dma_start(fixfull[:], fix0[:])

            # touch t_sb on the PE first (single-wait discipline for LW)
            ps_warm = pspool.tile([1, 1], fp32, tag="warm")
            nc.tensor.matmul(ps_warm[:], t_sb[0:1, 0, 0:1], t_sb[0:1, 0, 0:1],
                             start=True, stop=True)

            def conv_iter(u_in):
                """One Richardson step: returns (psum, u_next)."""
                ps = pspool.tile([ROWS, W], fp32, tag="ps")
                for b0 in (0, 512):
                    b1 = b0 + 512
                    nc.tensor.matmul(ps[:, b0:b1], t_sb[:, 1], u_in[:, b0:b1],
                                     start=True, stop=False)
                    if b0 == 0:
                        nc.tensor.matmul(ps[:, 1:512], t_sb[:, 0], u_in[:, 0:511],
                                         start=False, stop=False)
                        nc.tensor.matmul(ps[:, 0:512], t_sb[:, 2], u_in[:, 1:513],
                                         start=False, stop=True)
                    else:
                        nc.tensor.matmul(ps[:, 512:1024], t_sb[:, 0],
                                         u_in[:, 511:1023], start=False, stop=False)
                        nc.tensor.matmul(ps[:, 512:1023], t_sb[:, 2],
                                         u_in[:, 513:1024], start=False, stop=True)
                u_nx = upool.tile([ROWS, W], fp32, tag="u")
                nc.vector.scalar_tensor_tensor(
                    u_nx[:], ps[:], -float(DT), u0e_sb[:], Alu.mult, Alu.add)
                nc.vector.tensor_tensor(u_nx[:], u_nx[:], fixfull[:], Alu.add)
                return ps, u_nx

            def fix_from(h_top, h_bot):
                """3-tap FIR of packed halo rows -> fixfull rows 0/127.
                h_top/h_bot are [128, 10] APs; taps are already * -DT."""
                fpk = fpool.tile([128, 2, 8], fp32, tag="fpk")
                for side, h in ((0, h_top), (1, h_bot)):
                    tb = 3 * side
                    nc.vector.tensor_scalar_mul(
                        fpk[:, side], h[:, 0:8], taps_sb[:, tb:tb + 1])
                    nc.vector.scalar_tensor_tensor(
                        fpk[:, side], h[:, 1:9], taps_sb[:, tb + 1:tb + 2],
                        fpk[:, side], Alu.mult, Alu.add)
                    nc.vector.scalar_tensor_tensor(
                        fpk[:, side], h[:, 2:10], taps_sb[:, tb + 2:tb + 3],
                        fpk[:, side], Alu.mult, Alu.add)
                nc.sync.dma_start(fixfull[0:1, 0:512], fpk[0:64, 0])
                nc.scalar.dma_start(fixfull[0:1, 512:1024], fpk[64:128, 0])
                nc.sync.dma_start(fixfull[ROWS - 1:ROWS, 0:512], fpk[0:64, 1])
                nc.scalar.dma_start(fixfull[ROWS - 1:ROWS, 512:1024],
                                    fpk[64:128, 1])

            psum_last = None
            u_prev = None
            done = False

            for b in range(n_blocks):
                k = 2 * b
                if b > 0:
                    # gathered depth-2 halo of u_k: select the six roles
                    agout_d = agouts[(b - 1) % 2]
                    halo = halopool.tile([128, 32, 10], fp32, tag="halo")
                    ag_t = agout_d[:].tensor
                    ag_off = agout_d[:].offset
                    nc.sync.dma_start(
                        halo[:, 0:16],
                        bass.AP(ag_t, ag_off, [(8, 128), (1026, 16), (1, 10)]))
                    nc.scalar.dma_start(
                        halo[:, 16:32],
                        bass.AP(ag_t, ag_off + 16 * 1026,
                                [(8, 128), (1026, 16), (1, 10)]))
                    hv = halo[:].transpose([0, 2, 1])  # [128, 10, 32] view
                    hsel = fpool.tile([128, 6, 10], fp32, tag="hsel")
                    for role in range(6):
                        htmp = halopool.tile([128, 10, 32], fp32, tag="htmp")
                        nc.vector.tensor_tensor(htmp[:], hv, msel_sb[:, role],
                                                Alu.mult)
                        nc.vector.tensor_reduce(hsel[:, role], htmp[:],
                                                mybir.AxisListType.X, Alu.add)
                    # fix rows for iteration k  (u_k[-1] -> top, u_k[128] -> bot)
                    fix_from(hsel[:, 1], hsel[:, 4])

                ps, u_next = conv_iter(u_cur)
                if k == n_iters - 1:
                    psum_last, u_prev, u_cur = ps, u_cur, u_next
                    done = True
                    break

                # ---- halo rows of u_{k+1} computed locally (packed form) ----
                if b == 0:
                    er = halopool.tile([128, 2, 10], fp32, tag="er")
                    nc.scalar.dma_start(
                        er[:], bass.AP(est0[:].tensor, 0,
                                       [(8, 128), (1026, 2), (1, 10)]))
                else:
                    acc = fpool.tile([128, 2, 8], fp32, tag="acc")
                    for side, roles in ((0, (0, 1, 2)), (1, (3, 4, 5))):
                        first = True
                        for w_row, role in zip((0, 1, 2), roles):
                            for bc_ in range(3):
                                scal = taps9_sb[:, 3 * w_row + bc_:
                                                3 * w_row + bc_ + 1]
                                src = hsel[:, role, bc_:bc_ + 8]
                                if first:
                                    nc.vector.tensor_scalar_mul(
                                        acc[:, side], src, scal)
                                    first = False
                                else:
                                    nc.vector.scalar_tensor_tensor(
                                        acc[:, side], src, scal, acc[:, side],
                                        Alu.mult, Alu.add)
                    epk = fpool.tile([128, 2, 8], fp32, tag="epk")
                    nc.vector.scalar_tensor_tensor(
                        epk[:, 0], acc[:, 0], -float(DT), u0x_sb[:, 0],
                        Alu.mult, Alu.add)
                    nc.vector.scalar_tensor_tensor(
                        epk[:, 1], acc[:, 1], -float(DT), u0x_sb[:, 1],
                        Alu.mult, Alu.add)
                    estg = estgs[b % 2]
                    nc.sync.dma_start(estg[0:1, 1:1025], epk[:, 0])
                    nc.scalar.dma_start(estg[1:2, 1:1025], epk[:, 1])
                    er = halopool.tile([128, 2, 10], fp32, tag="er")
                    nc.scalar.dma_start(
                        er[:], bass.AP(estg[:].tensor, estg[:].offset,
                                       [(8, 128), (1026, 2), (1, 10)]))
                # fix rows for iteration k+1
                fix_from(er[:, 0], er[:, 1])

                ps2b, u_next2 = conv_iter(u_next)
                if k + 1 == n_iters - 1:
                    psum_last, u_prev, u_cur = ps2b, u_next, u_next2
                    done = True
                    break

                # ---- pack + AllGather of u_{k+2}'s four edge rows ----
                agin_d = agins[b % 2]
                nc.sync.dma_start(agin_d[0:2, 1:1025], u_next2[0:2, :])
                nc.scalar.dma_start(agin_d[2:4, 1:1025],
                                    u_next2[ROWS - 2:ROWS, :])
                nc.gpsimd.collective_compute(
                    "AllGather",
                    Alu.bypass,
                    replica_groups=[list(range(N_CORES))],
                    ins=[agin_d[:].opt()],
                    outs=[agouts[b % 2][:].opt()],
                )
                u_cur = u_next2

            assert done

            # ---- final residual norm: res = (u_prev + DT*psum) - u0eff ----
            tt = finpool.tile([ROWS, W], fp32, tag="tt")
            nc.vector.scalar_tensor_tensor(
                tt[:], psum_last[:], float(DT), u_prev[:], Alu.mult, Alu.add)
            nc.vector.scalar_tensor_tensor(
                tt[:], fixfull[:], -1.0, tt[:], Alu.mult, Alu.add)
            res = finpool.tile([ROWS, W], fp32, tag="res")
            nc.vector.tensor_tensor(res[:], tt[:], u0e_sb[:], Alu.subtract)
            sq = finpool.tile([ROWS, W], fp32, tag="sq")
            part = finpool.tile([128, 1], fp32, tag="part")
            nc.vector.tensor_tensor(sq[:], res[:], res[:], Alu.mult)
            nc.vector.tensor_reduce(part[:], sq[:], mybir.AxisListType.X, Alu.add)
            ps2 = pspool.tile([1, 1], fp32, tag="ps2")
            nc.tensor.matmul(ps2[:], ones[:], part[:], start=True, stop=True)
            nsq = finpool.tile([1, 1], fp32, tag="nsq")
            nc.vector.tensor_copy(nsq[:], ps2[:])
            nc.sync.dma_start(arin_d[0:1, 0:1], nsq[:])
            nc.gpsimd.collective_compute(
                "AllReduce",
                Alu.add,
                replica_groups=[list(range(N_CORES))],
                ins=[arin_d[:].opt()],
                outs=[arout_d[:].opt()],
            )
            gn = finpool.tile([1, 1], fp32, tag="gn")
            nc.sync.dma_start(gn[:], arout_d[0:1, 0:1])
            fres = finpool.tile([1, 1], fp32, tag="fres")
            nc.scalar.sqrt(fres[:], gn[:])

            # ---- outputs ----
            nc.sync.dma_start(u_out[:], u_cur[:])
            nc.sync.dma_start(res_out[:], fres[:])

    nc.finalize()
    return nc


def _host_prep(u_old, w1, w2, w3, b1, b2, b3):
    """Prepare per-core input dicts (all host-side numpy, f32)."""
    u0 = np.asarray(u_old, dtype=np.float32).reshape(H, W)
    w2 = np.asarray(w2, dtype=np.float32).reshape(3, 3)
    w3 = np.asarray(w3, dtype=np.float32).reshape(3, 3)
    bc = np.float32(np.asarray(b2, dtype=np.float32).reshape(()) +
                    np.asarray(b3, dtype=np.float32).reshape(()))
    wc = (w2 + w3).astype(np.float32)

    u0eff = (u0 - DT * bc).astype(np.float32)

    # banded row-mixing matrices, one per column offset b:
    # T_b[i_in, i_out] = wc[i_in - i_out + 1, b]
    tmats = []
    for b in range(3):
        T = (np.diag(np.full(127, wc[0, b]), 1) +
             np.diag(np.full(128, wc[1, b]), 0) +
             np.diag(np.full(127, wc[2, b]), -1)).astype(np.float32)
        tmats.append(T)
    tmat = np.stack(tmats, axis=1).reshape(128, 3 * 128)  # [i_in, (b, i_out)]

    taps = np.zeros((128, 6), dtype=np.float32)
    taps[:, 0:3] = (-DT) * wc[0, :]
    taps[:, 3:6] = (-DT) * wc[2, :]
    taps9 = np.broadcast_to(wc.reshape(1, 9), (128, 9)).astype(np.float32).copy()

    def fir(h, r):  # 3-tap FIR of a row with wc[r, :] (f32, zero-padded)
        hp = np.zeros(W + 2, dtype=np.float32)
        hp[1:-1] = h
        return (wc[r, 0] * hp[0:W] + wc[r, 1] * hp[1:W + 1] +
                wc[r, 2] * hp[2:W + 2]).astype(np.float32)

    def row(i):  # u0 row with zero padding outside the domain
        if 0 <= i < H:
            return u0[i]
        return np.zeros(W, dtype=np.float32)

    def u1_row(i):  # u_1[i] = u0eff[i] - DT * conv(u0)[i]  (f32 host math)
        conv = fir(row(i - 1), 0) + fir(row(i), 1) + fir(row(i + 1), 2)
        base = u0eff[i] if 0 <= i < H else np.zeros(W, dtype=np.float32)
        return (base - DT * conv).astype(np.float32)

    def reshape_pk(rowvec):  # [1024] -> packed [128, 8]
        return rowvec.reshape(128, 8)

    in_maps = []
    for c in range(N_CORES):
        r0, r1 = c * ROWS, (c + 1) * ROWS
        htop = row(r0 - 1)
        hbot = row(r1)
        fix0 = np.zeros((ROWS, W), dtype=np.float32)
        fix0[0] = (-DT) * fir(htop, 0)
        fix0[ROWS - 1] = (-DT) * fir(hbot, 2)

        est0 = np.zeros((2, 1026), dtype=np.float32)
        if c > 0:
            est0[0, 1:1025] = u1_row(r0 - 1)
        if c < N_CORES - 1:
            est0[1, 1:1025] = u1_row(r1)

        u0x = np.zeros((128, 16), dtype=np.float32)
        if c > 0:
            u0x[:, 0:8] = reshape_pk(u0eff[r0 - 1])
        if c < N_CORES - 1:
            u0x[:, 8:16] = reshape_pk(u0eff[r1])

        # role masks over the 32 AG rows (AG row 4i+s = core i's
        # {r0, r1, r126, r127}[s])
        msel_m = np.zeros((6, 10, 32), dtype=np.float32)
        if c > 0:
            msel_m[0, :, 4 * (c - 1) + 2] = 1.0   # u_k[-2]
            msel_m[1, :, 4 * (c - 1) + 3] = 1.0   # u_k[-1]
            msel_m[2, :, 4 * c + 0] = 1.0         # u_k[0] (ext-row input only;
        if c < N_CORES - 1:                       # zero on the edge cores so
            msel_m[3, :, 4 * c + 3] = 1.0         # the ghost rows stay 0)
            msel_m[4, :, 4 * (c + 1) + 0] = 1.0   # u_k[128]
            msel_m[5, :, 4 * (c + 1) + 1] = 1.0   # u_k[129]

        in_maps.append({
            "u_init": np.ascontiguousarray(u0[r0:r1]),
            "u0eff": np.ascontiguousarray(u0eff[r0:r1]),
            "fix0": fix0,
            "est0": est0,
            "zz": np.zeros((2, 1026), dtype=np.float32),
            "onesp": np.ones((128, 1), dtype=np.float32),
            "tmat": tmat,
            "taps": taps,
            "taps9": taps9,
            "u0ext": u0x,
            "msel": np.broadcast_to(
                msel_m.reshape(1, 6 * 10 * 32), (128, 6 * 10 * 32)).copy(),
        })
    return in_maps


def kernel(u_old, w1, w2, w3, b1, b2, b3):
    from concourse.bass_utils import run_bass_kernel_spmd

    n_iters = MAX_ITER
    if n_iters not in _GRAPH_CACHE:
        _GRAPH_CACHE[n_iters] = _build_graph(n_iters)
    nc = _GRAPH_CACHE[n_iters]

    in_maps = _host_prep(u_old, w1, w2, w3, b1, b2, b3)
    out = run_bass_kernel_spmd(nc, in_maps, core_ids=list(range(N_CORES)))
    results = out.results

    u_full = np.concatenate([results[c]["u_out"] for c in range(N_CORES)], axis=0)
    u_full = u_full.reshape(1, 1, H, W).astype(np.float32)
    final_res = np.float32(results[0]["res_out"].reshape(())[()])
    return u_full, np.asarray(final_res, dtype=np.float32).reshape(())
